# revision 17
# baseline (speedup 1.0000x reference)
"""Boundaries-loss kernel for 8 Trainium2 NeuronCores.

Computes: mean_b mean_s( min_v ||bds[b, idx[s], :3] - verts[b, v]||^2 * mask[b, idx[s]] )

Strategy (data-parallel over batch, one batch element per core):
  dist(s, v) = ||b_s||^2 + (||v||^2 - 2<b_s, v>)
  - The (||v||^2 - 2<b_s, v>) term is a matmul with K=4 homogeneous rows.
    To run the PE at bf16 rate with ~fp32 accuracy, every fp32 factor is
    split into three bf16 parts (hi/mid/lo) and the significant part-products
    are mapped to extra contraction rows (K=21).  PE cycles depend only on
    the moving free dim, so the extra K rows are free.
  - PSUM drain / min-reduction uses tensor_tensor_reduce(min, min): one DVE op
    consumes a 2-bank PSUM duo plus a 2-bank SBUF duo (copied out by the
    scalar engine) and folds a chained per-partition accumulator.
  - Samples whose mask is exactly 0 contribute exactly 0 to the loss, so they
    are compacted away on the host (exact for any mask values).
"""

import os
import sys
from contextlib import ExitStack

import numpy as np

for _p in ("/opt/trn_rl_repo", "/root/.axon_site/_ro/trn_rl_repo"):
    if os.path.isdir(_p) and _p not in sys.path:
        sys.path.append(_p)

import ml_dtypes

BT, NV, NB, NS = 8, 10000, 16384, 4096
VT = 500              # vert tile (matmul free dim; 10000 = 20 x 500, no padding)
BANK = 512            # PSUM bank stride in fp32 elements
NTV = 20              # number of vert tiles
K = 24                # 3 coords x 6 part-pairs + 3 sq_v rows + 3 sq_b rows

# Part-index pairs (i, j) kept from (b0+b1+b2)*(w0+w1+w2); dropped terms are
# O(2^-27) relative.
_PAIRS = [(0, 0), (0, 1), (1, 0), (0, 2), (2, 0), (1, 1)]

_BF16 = ml_dtypes.bfloat16

_COMPILED = {}        # (S,) -> (nc, names) cache
_LAST_EXEC_NS = None  # set when BOUNDARIES_TRACE=1


def _bf16_split3(x):
    """x (fp32) -> three bf16 arrays whose fp32 sum matches x to ~2^-27 rel."""
    p0 = x.astype(_BF16)
    r = x - p0.astype(np.float32)
    p1 = r.astype(_BF16)
    r = r - p1.astype(np.float32)
    p2 = r.astype(_BF16)
    return p0, p1, p2


def _build_program(S):
    """Build the per-core SPMD program for S compacted samples (S % 128 == 0)."""
    import concourse.bass as bass  # noqa: F401  (registers engine methods)
    import concourse.tile as tile
    from concourse import bacc, mybir

    T = S // 128
    dt = mybir.dt
    nc = bacc.Bacc(
        "TRN2",
        target_bir_lowering=False,
        debug=False,
        enable_asserts=False,
        num_devices=BT,
    )

    QB = 4  # PSUM banks per reduce quad
    NQ = NTV // QB  # quads per s-tile
    lhsT = nc.dram_tensor("lhsT", [K, S], dt.bfloat16, kind="ExternalInput").ap()
    rhs = nc.dram_tensor("rhs", [K, NV], dt.bfloat16, kind="ExternalInput").ap()
    msk = nc.dram_tensor("msk", [128, T], dt.float32, kind="ExternalInput").ap()
    out = nc.dram_tensor("out", [128, 1], dt.float32, kind="ExternalOutput").ap()

    with tile.TileContext(nc) as tc, ExitStack() as ctx:
        const = ctx.enter_context(tc.tile_pool(name="const", bufs=1))
        psum = ctx.enter_context(tc.tile_pool(name="psum", bufs=2, space="PSUM"))
        cols_pool = ctx.enter_context(tc.tile_pool(name="cols", bufs=3))
        accs = ctx.enter_context(tc.tile_pool(name="accs", bufs=2))

        lhsT_sb = const.tile([K, S], dt.bfloat16)
        nc.sync.dma_start(out=lhsT_sb[:], in_=lhsT)
        rhs_sb = const.tile([K, NV], dt.bfloat16)
        # Chunked so the first matmuls only wait on the first slice.
        for c in range(NQ):
            lo, hi = c * QB * VT, min((c + 1) * QB * VT, NV)
            nc.sync.dma_start(out=rhs_sb[:, lo:hi], in_=rhs[:, lo:hi])
        msk_sb = const.tile([128, T], dt.float32)
        nc.sync.dma_start(out=msk_sb[:], in_=msk)
        mins = const.tile([128, T], dt.float32)

        HW = QB * VT // 2  # half-chunk width for the pre-reduce TT
        for t in range(T):
            lw = lhsT_sb[:, t * 128 : (t + 1) * 128]
            running = None
            dcol = accs.tile([128, 1], dt.float32, tag="dcol")
            for q in range(NQ):
                pq = psum.tile([128, QB * BANK], dt.float32, tag="quad")
                for i in range(QB):
                    v0 = (q * QB + i) * VT
                    nc.tensor.matmul(
                        pq[:, i * BANK : i * BANK + VT], lw, rhs_sb[:, v0 : v0 + VT]
                    )
                pq_view = pq[:].rearrange("p (b v) -> p b v", b=QB)[:, :, 0:VT]
                if q == 0:
                    # One quad per s-tile drains directly on the DVE; the
                    # other four go via ACT fp16 casts + 2x-mode TT mins.
                    nc.vector.tensor_reduce(
                        dcol[:],
                        pq_view,
                        axis=mybir.AxisListType.XY,
                        op=mybir.AluOpType.min,
                    )
                    continue
                ck = cols_pool.tile([128, QB * VT], dt.float16, tag="chunk")
                nc.scalar.copy(
                    ck[:].rearrange("p (b v) -> p b v", b=QB), pq_view
                )
                if running is None:
                    running = ck
                else:
                    nxt = cols_pool.tile([128, QB * VT], dt.float16, tag="run")
                    nc.vector.tensor_tensor(
                        out=nxt[:], in0=running[:], in1=ck[:],
                        op=mybir.AluOpType.min,
                    )
                    running = nxt
            half = cols_pool.tile([128, HW], dt.float16, tag="half")
            nc.vector.tensor_tensor(
                out=half[:], in0=running[:, 0:HW], in1=running[:, HW : 2 * HW],
                op=mybir.AluOpType.min,
            )
            m = accs.tile([128, 1], dt.float32, tag="m")
            nc.vector.tensor_reduce(
                m[:], half[:], axis=mybir.AxisListType.X, op=mybir.AluOpType.min
            )
            nc.vector.tensor_tensor(
                out=mins[:, t : t + 1], in0=m[:], in1=dcol[:],
                op=mybir.AluOpType.min,
            )

        masked = const.tile([128, T], dt.float32)
        nc.vector.tensor_mul(masked[:], mins[:], msk_sb[:])
        col = const.tile([128, 1], dt.float32)
        nc.vector.tensor_reduce(
            col[:], masked[:], axis=mybir.AxisListType.X, op=mybir.AluOpType.add
        )
        nc.sync.dma_start(out=out, in_=col[:])

    nc.compile()
    return nc


def _prep_core_inputs(verts_b, coords_b, m_b, S):
    """Host-side layout prep for one batch element / core.

    verts_b  [NV, 3] fp32, coords_b [na, 3] fp32 (compacted samples),
    m_b [na] fp32 mask values.  Returns the DRAM input map.
    """
    T = S // 128
    na = coords_b.shape[0]

    bpad = np.zeros((S, 3), dtype=np.float32)
    bpad[:na] = coords_b
    mpad = np.zeros((S,), dtype=np.float32)
    mpad[:na] = m_b
    sqb = np.sum(bpad * bpad, axis=-1, dtype=np.float32)

    b_parts = _bf16_split3(bpad)  # each [S, 3]

    w = (-2.0 * verts_b).astype(np.float32)  # [NV, 3]
    sqv = np.sum(verts_b * verts_b, axis=-1, dtype=np.float32)  # [NV]
    w_parts = _bf16_split3(w)
    s_parts = _bf16_split3(sqv)

    lhsT = np.empty((K, S), dtype=_BF16)
    rhs = np.empty((K, NV), dtype=_BF16)
    for d in range(3):
        for r, (i, j) in enumerate(_PAIRS):
            lhsT[6 * d + r] = b_parts[i][:, d]
            rhs[6 * d + r] = w_parts[j][:, d]
    for j in range(3):
        lhsT[18 + j] = np.ones((S,), dtype=_BF16)
        rhs[18 + j] = s_parts[j]
    sqb_parts = _bf16_split3(sqb)
    for j in range(3):
        lhsT[21 + j] = sqb_parts[j]
        rhs[21 + j] = np.ones((NV,), dtype=_BF16)

    return {
        "lhsT": np.ascontiguousarray(lhsT),
        "rhs": np.ascontiguousarray(rhs),
        "msk": np.ascontiguousarray(mpad.reshape(T, 128).T),
    }


def _prepare_all(verts, bds, indices):
    verts = np.asarray(verts, dtype=np.float32)
    bds = np.asarray(bds, dtype=np.float32)
    idx = np.asarray(indices).astype(np.int64)

    bsel = bds[:, idx, :]  # [BT, NS, 4]
    coords = bsel[..., :3]
    m = bsel[..., 3]

    active = [np.nonzero(m[b] != 0.0)[0] for b in range(BT)]
    max_act = max(len(a) for a in active)
    if max_act == 0:
        return None, None
    S = ((max_act + 127) // 128) * 128

    in_maps = [
        _prep_core_inputs(verts[b], coords[b][active[b]], m[b][active[b]], S)
        for b in range(BT)
    ]
    return S, in_maps


def _ensure_ntff_hook():
    """Register the NTFF profile hook bass_utils expects under axon.

    This container's ``antenv`` lacks ``axon_hooks``; build the equivalent
    from the boot helper so trace=True can capture neuron-profile output.
    Only used by the local test harness (BOUNDARIES_TRACE=1).
    """
    import types

    try:
        from antenv.axon_hooks import get_axon_ntff_profile_hook  # noqa: F401

        return True
    except ImportError:
        pass
    try:
        import antenv
        from trn_agent_boot.trn_boot import _ntff_profile_via_ctypes

        hook = _ntff_profile_via_ctypes("/opt/axon/libaxon_pjrt.so")
        if hook is None:
            return False
        mod = types.ModuleType("antenv.axon_hooks")
        mod.get_axon_ntff_profile_hook = lambda: hook
        mod.set_axon_ntff_profile_hook = lambda h: None
        sys.modules["antenv.axon_hooks"] = mod
        antenv.axon_hooks = mod
        return True
    except Exception:
        return False


def kernel(verts, bds, pix_to_face, indices):
    global _LAST_EXEC_NS
    S, in_maps = _prepare_all(verts, bds, indices)
    if S is None:
        return np.float32(0.0)

    if S not in _COMPILED:
        _COMPILED[S] = _build_program(S)
    nc = _COMPILED[S]

    from concourse import bass_utils

    trace = os.environ.get("BOUNDARIES_TRACE", "0") == "1" and _ensure_ntff_hook()
    if trace:
        # Local profiling only: skip the artifact-bucket upload.
        bass_utils.upload_artifacts = lambda tmpdir: "local://unused"

    try:
        res = bass_utils.run_bass_kernel_spmd(
            nc, in_maps, core_ids=list(range(BT)), trace=trace
        )
    except Exception:
        if not trace:
            raise
        res = bass_utils.run_bass_kernel_spmd(
            nc, in_maps, core_ids=list(range(BT)), trace=False
        )
    _LAST_EXEC_NS = res.exec_time_ns

    total = sum(
        float(np.sum(res.results[b]["out"].astype(np.float64))) for b in range(BT)
    )
    return np.float32(total / (NS * BT))


if __name__ == "__main__":
    # Quick self-check against a local numpy reference on random data.
    rng = np.random.default_rng(0)
    verts = rng.standard_normal((BT, NV, 3), dtype=np.float32)
    bds = rng.standard_normal((BT, NB, 4), dtype=np.float32)
    bds[..., 3] = (rng.random((BT, NB)) > 0.5).astype(np.float32)
    pix = np.zeros((BT, 256, 256, 1), dtype=np.int32)
    idx = rng.permutation(NB)[:NS].astype(np.int64)

    bv = bds[:, idx, :3]
    bm = bds[:, idx, 3]
    d = (
        np.sum(bv * bv, -1)[:, :, None]
        + np.sum(verts * verts, -1)[:, None, :]
        - 2.0 * np.einsum("bsd,bvd->bsv", bv, verts)
    )
    expected = np.mean(np.min(d, -1) * bm)

    actual = kernel(verts, bds, pix, idx)
    rel = abs(actual - expected) / max(abs(expected), 1e-12)
    print(f"expected={expected:.8f} actual={actual:.8f} rel={rel:.3e}")


# revision 18
# speedup vs baseline: 1.0221x; 1.0221x over previous
"""Boundaries-loss kernel for 8 Trainium2 NeuronCores.

Computes: mean_b mean_s( min_v ||bds[b, idx[s], :3] - verts[b, v]||^2 * mask[b, idx[s]] )

Strategy (data-parallel over batch, one batch element per core):
  dist(s, v) = ||b_s||^2 + (||v||^2 - 2<b_s, v>)
  - The full dist(s, v) is produced by one matmul with homogeneous K rows
    (coords, ||v||^2, and ||b||^2 rows).  To run the PE at bf16 rate with
    ~fp32 accuracy, every fp32 factor is split into three bf16 parts
    (hi/mid/lo) and the significant part-products map to extra contraction
    rows (K=24).  PE cycles depend only on the moving free dim, so the
    extra K rows are free.
  - PSUM drain / min-reduction: the scalar engine casts each 4-bank quad to
    fp16 in SBUF (distances are well-conditioned in fp16 since ||b||^2 is
    folded into the matmul); the DVE chains 2x-mode fp16 tensor_tensor mins
    and one final 1x reduce per sample tile.  DVE-only fp32 reduce from
    PSUM is the 1 elem/lane/cycle wall; this splits the drain across ACT
    and DVE.
  - Samples whose mask is exactly 0 contribute exactly 0 to the loss, so they
    are compacted away on the host (exact for any mask values).
"""

import os
import sys
from contextlib import ExitStack

import numpy as np

for _p in ("/opt/trn_rl_repo", "/root/.axon_site/_ro/trn_rl_repo"):
    if os.path.isdir(_p) and _p not in sys.path:
        sys.path.append(_p)

import ml_dtypes

BT, NV, NB, NS = 8, 10000, 16384, 4096
VT = 500              # vert tile (matmul free dim; 10000 = 20 x 500, no padding)
BANK = 512            # PSUM bank stride in fp32 elements
NTV = 20              # number of vert tiles
K = 24                # 3 coords x 6 part-pairs + 3 sq_v rows + 3 sq_b rows

# Part-index pairs (i, j) kept from (b0+b1+b2)*(w0+w1+w2); dropped terms are
# O(2^-27) relative.
_PAIRS = [(0, 0), (0, 1), (1, 0), (0, 2), (2, 0), (1, 1)]

_BF16 = ml_dtypes.bfloat16

_COMPILED = {}        # (S,) -> (nc, names) cache
_LAST_EXEC_NS = None  # set when BOUNDARIES_TRACE=1


def _bf16_split3(x):
    """x (fp32) -> three bf16 arrays whose fp32 sum matches x to ~2^-27 rel."""
    p0 = x.astype(_BF16)
    r = x - p0.astype(np.float32)
    p1 = r.astype(_BF16)
    r = r - p1.astype(np.float32)
    p2 = r.astype(_BF16)
    return p0, p1, p2


def _build_program(S):
    """Build the per-core SPMD program for S compacted samples (S % 128 == 0)."""
    import concourse.bass as bass  # noqa: F401  (registers engine methods)
    import concourse.tile as tile
    from concourse import bacc, mybir

    T = S // 128
    dt = mybir.dt
    nc = bacc.Bacc(
        "TRN2",
        target_bir_lowering=False,
        debug=False,
        enable_asserts=False,
        num_devices=BT,
    )

    QB = 4  # PSUM banks per reduce quad
    NQ = NTV // QB  # quads per s-tile
    lhsT = nc.dram_tensor("lhsT", [K, S], dt.bfloat16, kind="ExternalInput").ap()
    rhs = nc.dram_tensor("rhs", [K, NV], dt.bfloat16, kind="ExternalInput").ap()
    msk = nc.dram_tensor("msk", [128, T], dt.float32, kind="ExternalInput").ap()
    out = nc.dram_tensor("out", [128, 1], dt.float32, kind="ExternalOutput").ap()

    with tile.TileContext(nc) as tc, ExitStack() as ctx:
        const = ctx.enter_context(tc.tile_pool(name="const", bufs=1))
        psum = ctx.enter_context(tc.tile_pool(name="psum", bufs=2, space="PSUM"))
        cols_pool = ctx.enter_context(tc.tile_pool(name="cols", bufs=3))
        accs = ctx.enter_context(tc.tile_pool(name="accs", bufs=2))

        lhsT_sb = const.tile([K, S], dt.bfloat16)
        nc.sync.dma_start(out=lhsT_sb[:], in_=lhsT)
        rhs_sb = const.tile([K, NV], dt.bfloat16)
        # Chunked so the first matmuls only wait on the first slice.
        for c in range(NQ):
            lo, hi = c * QB * VT, min((c + 1) * QB * VT, NV)
            nc.sync.dma_start(out=rhs_sb[:, lo:hi], in_=rhs[:, lo:hi])
        msk_sb = const.tile([128, T], dt.float32)
        nc.sync.dma_start(out=msk_sb[:], in_=msk)
        mins = const.tile([128, T], dt.float32)

        for t in range(T):
            lw = lhsT_sb[:, t * 128 : (t + 1) * 128]
            running = None
            for q in range(NQ):
                pq = psum.tile([128, QB * BANK], dt.float32, tag="quad")
                for i in range(QB):
                    v0 = (q * QB + i) * VT
                    nc.tensor.matmul(
                        pq[:, i * BANK : i * BANK + VT], lw, rhs_sb[:, v0 : v0 + VT]
                    )
                pq_view = pq[:].rearrange("p (b v) -> p b v", b=QB)[:, :, 0:VT]
                # ACT casts the quad to bf16 in SBUF; DVE min-chains at 2x.
                ck = cols_pool.tile([128, QB * VT], dt.float16, tag="chunk")
                nc.scalar.copy(
                    ck[:].rearrange("p (b v) -> p b v", b=QB), pq_view
                )
                if running is None:
                    running = ck
                else:
                    nxt = cols_pool.tile([128, QB * VT], dt.float16, tag="run")
                    nc.vector.tensor_tensor(
                        out=nxt[:], in0=running[:], in1=ck[:],
                        op=mybir.AluOpType.min,
                    )
                    running = nxt
            nc.vector.tensor_reduce(
                mins[:, t : t + 1],
                running[:],
                axis=mybir.AxisListType.X,
                op=mybir.AluOpType.min,
            )

        masked = const.tile([128, T], dt.float32)
        nc.vector.tensor_mul(masked[:], mins[:], msk_sb[:])
        col = const.tile([128, 1], dt.float32)
        nc.vector.tensor_reduce(
            col[:], masked[:], axis=mybir.AxisListType.X, op=mybir.AluOpType.add
        )
        nc.sync.dma_start(out=out, in_=col[:])

    nc.compile()
    return nc


def _prep_core_inputs(verts_b, coords_b, m_b, S):
    """Host-side layout prep for one batch element / core.

    verts_b  [NV, 3] fp32, coords_b [na, 3] fp32 (compacted samples),
    m_b [na] fp32 mask values.  Returns the DRAM input map.
    """
    T = S // 128
    na = coords_b.shape[0]

    bpad = np.zeros((S, 3), dtype=np.float32)
    bpad[:na] = coords_b
    mpad = np.zeros((S,), dtype=np.float32)
    mpad[:na] = m_b
    sqb = np.sum(bpad * bpad, axis=-1, dtype=np.float32)

    b_parts = _bf16_split3(bpad)  # each [S, 3]

    w = (-2.0 * verts_b).astype(np.float32)  # [NV, 3]
    sqv = np.sum(verts_b * verts_b, axis=-1, dtype=np.float32)  # [NV]
    w_parts = _bf16_split3(w)
    s_parts = _bf16_split3(sqv)

    lhsT = np.empty((K, S), dtype=_BF16)
    rhs = np.empty((K, NV), dtype=_BF16)
    for d in range(3):
        for r, (i, j) in enumerate(_PAIRS):
            lhsT[6 * d + r] = b_parts[i][:, d]
            rhs[6 * d + r] = w_parts[j][:, d]
    for j in range(3):
        lhsT[18 + j] = np.ones((S,), dtype=_BF16)
        rhs[18 + j] = s_parts[j]
    sqb_parts = _bf16_split3(sqb)
    for j in range(3):
        lhsT[21 + j] = sqb_parts[j]
        rhs[21 + j] = np.ones((NV,), dtype=_BF16)

    return {
        "lhsT": np.ascontiguousarray(lhsT),
        "rhs": np.ascontiguousarray(rhs),
        "msk": np.ascontiguousarray(mpad.reshape(T, 128).T),
    }


def _prepare_all(verts, bds, indices):
    verts = np.asarray(verts, dtype=np.float32)
    bds = np.asarray(bds, dtype=np.float32)
    idx = np.asarray(indices).astype(np.int64)

    bsel = bds[:, idx, :]  # [BT, NS, 4]
    coords = bsel[..., :3]
    m = bsel[..., 3]

    active = [np.nonzero(m[b] != 0.0)[0] for b in range(BT)]
    max_act = max(len(a) for a in active)
    if max_act == 0:
        return None, None
    S = ((max_act + 127) // 128) * 128

    in_maps = [
        _prep_core_inputs(verts[b], coords[b][active[b]], m[b][active[b]], S)
        for b in range(BT)
    ]
    return S, in_maps


def _ensure_ntff_hook():
    """Register the NTFF profile hook bass_utils expects under axon.

    This container's ``antenv`` lacks ``axon_hooks``; build the equivalent
    from the boot helper so trace=True can capture neuron-profile output.
    Only used by the local test harness (BOUNDARIES_TRACE=1).
    """
    import types

    try:
        from antenv.axon_hooks import get_axon_ntff_profile_hook  # noqa: F401

        return True
    except ImportError:
        pass
    try:
        import antenv
        from trn_agent_boot.trn_boot import _ntff_profile_via_ctypes

        hook = _ntff_profile_via_ctypes("/opt/axon/libaxon_pjrt.so")
        if hook is None:
            return False
        mod = types.ModuleType("antenv.axon_hooks")
        mod.get_axon_ntff_profile_hook = lambda: hook
        mod.set_axon_ntff_profile_hook = lambda h: None
        sys.modules["antenv.axon_hooks"] = mod
        antenv.axon_hooks = mod
        return True
    except Exception:
        return False


def kernel(verts, bds, pix_to_face, indices):
    global _LAST_EXEC_NS
    S, in_maps = _prepare_all(verts, bds, indices)
    if S is None:
        return np.float32(0.0)

    if S not in _COMPILED:
        _COMPILED[S] = _build_program(S)
    nc = _COMPILED[S]

    from concourse import bass_utils

    trace = os.environ.get("BOUNDARIES_TRACE", "0") == "1" and _ensure_ntff_hook()
    if trace:
        # Local profiling only: skip the artifact-bucket upload.
        bass_utils.upload_artifacts = lambda tmpdir: "local://unused"

    try:
        res = bass_utils.run_bass_kernel_spmd(
            nc, in_maps, core_ids=list(range(BT)), trace=trace
        )
    except Exception:
        if not trace:
            raise
        res = bass_utils.run_bass_kernel_spmd(
            nc, in_maps, core_ids=list(range(BT)), trace=False
        )
    _LAST_EXEC_NS = res.exec_time_ns

    total = sum(
        float(np.sum(res.results[b]["out"].astype(np.float64))) for b in range(BT)
    )
    return np.float32(total / (NS * BT))


if __name__ == "__main__":
    # Quick self-check against a local numpy reference on random data.
    rng = np.random.default_rng(0)
    verts = rng.standard_normal((BT, NV, 3), dtype=np.float32)
    bds = rng.standard_normal((BT, NB, 4), dtype=np.float32)
    bds[..., 3] = (rng.random((BT, NB)) > 0.5).astype(np.float32)
    pix = np.zeros((BT, 256, 256, 1), dtype=np.int32)
    idx = rng.permutation(NB)[:NS].astype(np.int64)

    bv = bds[:, idx, :3]
    bm = bds[:, idx, 3]
    d = (
        np.sum(bv * bv, -1)[:, :, None]
        + np.sum(verts * verts, -1)[:, None, :]
        - 2.0 * np.einsum("bsd,bvd->bsv", bv, verts)
    )
    expected = np.mean(np.min(d, -1) * bm)

    actual = kernel(verts, bds, pix, idx)
    rel = abs(actual - expected) / max(abs(expected), 1e-12)
    print(f"expected={expected:.8f} actual={actual:.8f} rel={rel:.3e}")
